# revision 26
# baseline (speedup 1.0000x reference)
"""Trainium2 Bass kernel for nn_CustomConvLayer (3x3-tap conv).

out[b,o,h,w] = sum_c sum_k x_pad[b,c,h+dh_k,w+dw_k] * weights[o,c,k]
x: [16,64,128,128] f32, weights: [128,64,9] f32 -> out: [16,128,128,128] f32

Strategy (8 NeuronCores, data-parallel over batch, 2 images/core), default
variant "q32r":
- Host pre-pads each image plane to 130x130, converts to bf16, and lays the
  two images of a core on SBUF partitions 0-63 (img0) and 64-127 (img1).
- TensorE runs FOUR concurrent K=32 row-tiled matmul streams (tile
  positions (0,0)/(32,0)/(64,0)/(96,0) = img x channel-half), issued
  round-robin so the 32-row sub-arrays compute concurrently (measured
  ~282ns per 4-matmul volley at N=512 vs 213ns for a single stream; the
  per-volley LDWEIGHTS pull-aheads hide behind the other tiles' streams).
  9 taps accumulate into separate PSUM banks per (image, c-half, 4-row
  group); tap shifts are pure AP offsets into the padded plane - no
  im2col, no data duplication. The two c-half banks are merged at
  evacuation: Activation copies bank A (f32->bf16 cast), DVE adds bank B.
- bf16 operands keep the full TensorE rate while halving HBM traffic on
  loads. Host upcasts the bf16 result to f32 (rel err ~4.7e-3 vs the
  fp32 reference; fp8 tested and rejected: e4m3 rel err 3.8e-2 > 2e-2).
- x is DMA-loaded in row-chunks (small first chunk) on the sync HWDGE
  ring so compute starts early; the x SBUF tile is double-buffered across
  rep-loop iterations. Stores ride the scalar HWDGE ring so chunk loads
  never queue behind ~8MB of store traffic, and the 6-deep output staging
  pool keeps the store WAR reuse distance past the ~2us HBM write-receipt
  latency (q32e->q32r: 90.2us -> 81.4us).
"""
import numpy as np

C, O, H, Wd, Wp, KT, NIMG, R, G = 64, 128, 128, 128, 130, 9, 2, 4, 32
NCORES = 8
TAPS = [(dh, dw) for dh in range(3) for dw in range(3)]

_CACHE = {}

VARIANT = "q32r"  # matmul-ordering experiment knob; see _build


def _dedup_ldweights(nc, mybir):
    """Remove InstLdweights that reload the exact weights already resident in
    the same PE tile position (no intervening load to that position), merging
    their dependency edges into the following matmult. The tile scheduler
    emits one InstLdweights per InstMatmult; for orderings that run several
    same-weight matmuls back-to-back at one position this drops the redundant
    64-cycle reloads that would otherwise serialize with the stream."""
    ndrop = 0
    for blk in nc.m.functions[0].blocks:
        insts = list(blk.instructions)
        resident = {}  # tile_position -> weights key
        pending = {}  # tile_position -> (ldw, key)
        drop = set()
        for inst in insts:
            if isinstance(inst, mybir.InstLdweights):
                pos = tuple(inst.tile_position or (0, 0))
                key = str(inst.ins[0])
                pending[pos] = (inst, key)
            elif isinstance(inst, mybir.InstMatmult):
                pos = tuple(inst.tile_position or (0, 0))
                if pos in pending:
                    ldw, key = pending.pop(pos)
                    if resident.get(pos) == key:
                        inst.merge_dependencies_from(ldw)
                        drop.add(ldw.name)
                    else:
                        resident[pos] = key
        if drop:
            ndrop += len(drop)
            for inst in insts:
                for t in list(inst.sync_dependency_names()):
                    if t in drop:
                        inst.try_remove_dependency(t)
                for t in list(inst.nosync_dependency_names()):
                    if t in drop:
                        inst.try_remove_dependency(t)
            blk.instructions = [i for i in insts if i.name not in drop]
    return ndrop


def _build(rep=1, variant=None):
    import os
    from concourse import bacc
    import concourse.mybir as mybir
    from concourse.tile import TileContext

    if variant is None:
        variant = os.environ.get("CONV_VARIANT", VARIANT)

    f32 = mybir.dt.float32
    bf16 = mybir.dt.bfloat16

    nc = bacc.Bacc()
    wcols = 6 * O if variant.startswith("dc6") else KT * O
    xp = nc.declare_dram_parameter("xp", [NIMG * C, Wp * Wp], bf16, isOutput=False)
    wp = nc.declare_dram_parameter("wp", [128, wcols], bf16, isOutput=False)
    out = nc.declare_dram_parameter("out", [NIMG, O, H, Wd], bf16, isOutput=True)

    # x row-chunks: small first chunk so group-0 matmuls start early;
    # the rest sized for DMA efficiency. Rows must sum to Wp=130.
    if variant == "q32rc":
        CHUNK_ROWS = [6, 60, 64]
    else:
        CHUNK_ROWS = [6] + [14] * 8 + [12]

    with TileContext(nc) as tc:
        with tc.tile_pool(name="xpool",
                          bufs=(4 if variant.startswith("dc6") else
                                3 if variant == "q32r3" else 2)) as xpool, \
             tc.tile_pool(name="wpool", bufs=1) as wpool, \
             tc.tile_pool(name="spool", bufs=6 if variant in ("q32r", "q32rc", "q32p", "q32x", "q32f", "q32r3") else 3) as spool, \
             tc.tile_pool(name="ps", bufs=8, space="PSUM") as pspool:
            wt = wpool.tile([128, wcols], bf16)
            # weights on the gpsimd (SWDGE) ring so they land in parallel
            # with the sync-ring x chunk loads
            nc.gpsimd.dma_start(out=wt[:], in_=wp[:])
            wv = wt[:].rearrange("p (k o) -> p k o", o=O)

            def body(it=0):
                if variant.startswith("dc6"):
                    # per-image dual-copy tiles: partitions 0-63 = padded
                    # plane (copyA), 64-127 = plane shifted down one row
                    # (copyB), both DMA'd from the same HBM buffer
                    xts = [xpool.tile([128, Wp * Wp], bf16, tag=f"xt{i}")
                           for i in range(NIMG)]
                    xv2 = [t[:].rearrange("p (r w) -> p r w", w=Wp)
                           for t in xts]
                    row = 0
                    for ck, nrows in enumerate(CHUNK_ROWS):
                        r0, r1 = row, row + nrows
                        r1b = min(r1, Wp - 1)  # copyB valid rows: 0..128
                        for img in range(NIMG):
                            cb = img * C
                            nc.sync.dma_start(
                                out=xts[img][0:C, r0 * Wp:r1 * Wp],
                                in_=xp[cb:cb + C, r0 * Wp:r1 * Wp])
                            if r1b > r0:
                                nc.sync.dma_start(
                                    out=xts[img][C:2 * C, r0 * Wp:r1b * Wp],
                                    in_=xp[cb:cb + C,
                                           r0 * Wp + Wp:r1b * Wp + Wp])
                        row += nrows
                else:
                    xt = xpool.tile([128, Wp * Wp], bf16, tag="xt")
                    xv = xt[:].rearrange("p (r w) -> p r w", w=Wp)
                    row = 0
                    for ck, nrows in enumerate(CHUNK_ROWS):
                        o0 = row * Wp
                        o1 = o0 + nrows * Wp
                        if variant not in ("pemm", "nold", "q32p") or ck == 0:
                            ldeng = nc.gpsimd if (variant == "q32x" and
                                                  ck % 2 == 1) else nc.sync
                            ldeng.dma_start(out=xt[:, o0:o1], in_=xp[:, o0:o1])
                        row += nrows

                def rhs_ap(img, g, t):
                    dh, dw = TAPS[t]
                    b = img * 64
                    h0 = g * R
                    return xv[b:b + 64, h0 + dh:h0 + dh + R, dw:dw + Wd]

                def lhs_ap(img, t):
                    return wv[img * 64:img * 64 + 64, t, :]

                st = [None, None]

                def evac(g, q, img, pst_t, last_block):
                    nc.vector.tensor_copy(st[img][:, q * R * Wd:(q + 1) * R * Wd],
                                          pst_t[:])
                    if variant in ("pemm", "nost"):
                        return
                    h0 = g * R
                    if last_block:
                        # final block: store per-group (128KB) so the
                        # post-compute DMA tail is short
                        nc.scalar.dma_start(out=out[img, :, h0:h0 + R, :],
                                            in_=st[img][:, q * R * Wd:(q + 1) * R * Wd])
                    elif q == 3:
                        hs = (g // 4) * 16
                        nc.scalar.dma_start(out=out[img, :, hs:hs + 16, :],
                                            in_=st[img][:])

                if variant == "q32":
                    # 4 row-quarter tile positions: (0,0) img0/cA, (32,0)
                    # img0/cB, (64,0) img1/cA, (96,0) img1/cB; each matmul
                    # contracts 32 channels, so its 32-row weight load (the
                    # per-position serial cost) halves vs 64-deep streams.
                    # Both c-halves of an image accumulate into the SAME PSUM
                    # bank: in-order issue keeps the cB stream >=32 cycles
                    # behind cA at every address, so the read-modify-write
                    # accumulations interleave safely; start/stop flags mark
                    # only the first/last writer of the bank.
                    for g in range(G):
                        q = g % 4
                        if q == 0:
                            st = [spool.tile([128, 4 * R * Wd], bf16, tag="st",
                                             name=f"st{g}_{i}") for i in range(NIMG)]
                        pst = [pspool.tile([128, R * Wd], f32, tag="ps",
                                           name=f"ps{g}_{i}") for i in range(NIMG)]
                        h0 = g * R
                        for t in range(KT):
                            dh, dw = TAPS[t]
                            for img in range(NIMG):
                                for hh in range(2):
                                    b = img * 64 + hh * 32
                                    rhs = xv[b:b + 32, h0 + dh:h0 + dh + R,
                                             dw:dw + Wd]
                                    lhsT = wv[b:b + 32, t, :]
                                    nc.tensor.matmul(pst[img][:], lhsT, rhs,
                                                     start=(t == 0 and hh == 0),
                                                     stop=(t == KT - 1 and hh == 1),
                                                     tile_position=(b, 0),
                                                     skip_group_check=True)
                        last_block = g >= G - 4
                        for img in range(NIMG):
                            evac(g, q, img, pst[img], last_block)
                elif variant == "q32f":
                    # phase-shifted c-halves: tile pair (hh=0) processes
                    # group s while (hh=1) finishes group s-1, both
                    # accumulating into the SAME bank per (group, img) but 9
                    # volleys apart - never concurrently (drain margin is one
                    # full volley, ~282ns >> array drain). Bank sees hh0 taps
                    # 0..8 then hh1 taps 0..8: start on the first, stop on
                    # the last, one DVE copy evacuates (no Act copy, no add).
                    pstm = {}
                    for s in range(G + 1):
                        if s % 4 == 1:
                            # st block for groups s-1 .. s+2 (evacs trail the
                            # hh0 stream by one slot)
                            st = [spool.tile([128, 4 * R * Wd], bf16, tag="st",
                                             name=f"st{s}_{i}") for i in range(NIMG)]
                        for i in range(NIMG):
                            if s < G:
                                pstm[(s, i)] = pspool.tile(
                                    [128, R * Wd], f32, tag="ps",
                                    name=f"ps{s}_{i}")
                        for t in range(KT):
                            dh, dw = TAPS[t]
                            for img in range(NIMG):
                                for hh in range(2):
                                    g = s - hh
                                    if g < 0 or g >= G:
                                        continue
                                    b = img * 64 + hh * 32
                                    h0 = g * R
                                    rhs = xv[b:b + 32, h0 + dh:h0 + dh + R,
                                             dw:dw + Wd]
                                    nc.tensor.matmul(
                                        pstm[(g, img)][:], wv[b:b + 32, t, :],
                                        rhs,
                                        start=(hh == 0 and t == 0),
                                        stop=(hh == 1 and t == KT - 1),
                                        tile_position=(b, 0),
                                        skip_group_check=True)
                        if s >= 1:
                            g = s - 1
                            last_block = g >= G - 4
                            for img in range(NIMG):
                                evac(g, g % 4, img, pstm.pop((g, img)),
                                     last_block)
                elif variant in ("q32e", "q32r", "q32rc", "q32p", "q32x", "q32r3"):
                    # 4 row-quarter positions, separate PSUM banks per
                    # c-half. Merge work split across engines: Activation
                    # copies bank A (with bf16 cast), DVE adds bank B.
                    # q32e: stores ride the sync ring (shared with loads);
                    # q32r/q32rc: stores move to the scalar HWDGE ring so
                    # iteration k+1's chunk loads don't queue behind ~8MB of
                    # stores, and spool bufs=6 keeps the st-buffer WAR reuse
                    # distance well past the ~2us HBM write-receipt latency.
                    store_eng = nc.sync if variant == "q32e" else nc.scalar
                    for g in range(G):
                        q = g % 4
                        if q == 0:
                            st = [spool.tile([128, 4 * R * Wd], bf16, tag="st",
                                             name=f"st{g}_{i}") for i in range(NIMG)]
                        pst = [[pspool.tile([128, R * Wd], f32, tag="ps",
                                            name=f"ps{g}_{i}_{h}") for h in range(2)]
                               for i in range(NIMG)]
                        h0 = g * R
                        for t in range(KT):
                            dh, dw = TAPS[t]
                            for img in range(NIMG):
                                for hh in range(2):
                                    b = img * 64 + hh * 32
                                    rhs = xv[b:b + 32, h0 + dh:h0 + dh + R,
                                             dw:dw + Wd]
                                    lhsT = wv[b:b + 32, t, :]
                                    nc.tensor.matmul(pst[img][hh][:], lhsT, rhs,
                                                     start=(t == 0),
                                                     stop=(t == KT - 1),
                                                     tile_position=(b, 0))
                        last_block = g >= G - 4
                        for img in range(NIMG):
                            dst = st[img][:, q * R * Wd:(q + 1) * R * Wd]
                            nc.scalar.copy(dst, pst[img][0][:])
                            nc.vector.tensor_add(dst, dst, pst[img][1][:])
                            if variant == "q32p":
                                continue
                            if last_block:
                                store_eng.dma_start(out=out[img, :, h0:h0 + R, :],
                                                    in_=dst)
                            elif q == 3:
                                hs = (g // 4) * 16
                                store_eng.dma_start(out=out[img, :, hs:hs + 16, :],
                                                    in_=st[img][:])
                elif variant in ("q32u", "q32ud"):
                    # q32e refined: 4 K=32 tile positions round-robin
                    # (concurrent sub-array streams), separate PSUM banks per
                    # c-half, merged q32e-style (Act copies bank A with bf16
                    # cast, DVE adds bank B); snake tap order between groups
                    # lets the dedup pass drop the 4 boundary ldweights of
                    # each adjacent group pair.
                    prev_mm = [None]

                    def cmm(*args, **kw):
                        bi = nc.tensor.matmul(*args, **kw)
                        if prev_mm[0] is not None:
                            bi.ins.add_dependency(
                                prev_mm[0], mybir.DependencyInfo.NO_SYNC_ONLY)
                        prev_mm[0] = bi.ins.name
                        return bi

                    for g in range(G):
                        q = g % 4
                        if q == 0:
                            st = [spool.tile([128, 4 * R * Wd], bf16, tag="st",
                                             name=f"st{g}_{i}") for i in range(NIMG)]
                        pst = [[pspool.tile([128, R * Wd], f32, tag="ps",
                                            name=f"ps{g}_{i}_{h}") for h in range(2)]
                               for i in range(NIMG)]
                        h0 = g * R
                        taps = range(KT) if g % 2 == 0 else range(KT - 1, -1, -1)
                        for ti, t in enumerate(taps):
                            dh, dw = TAPS[t]
                            for img in range(NIMG):
                                for hh in range(2):
                                    b = img * 64 + hh * 32
                                    rhs = xv[b:b + 32, h0 + dh:h0 + dh + R,
                                             dw:dw + Wd]
                                    lhsT = wv[b:b + 32, t, :]
                                    cmm(pst[img][hh][:], lhsT, rhs,
                                        start=(ti == 0), stop=(ti == KT - 1),
                                        tile_position=(b, 0))
                        last_block = g >= G - 4
                        for img in range(NIMG):
                            dst = st[img][:, q * R * Wd:(q + 1) * R * Wd]
                            nc.scalar.copy(dst, pst[img][0][:])
                            nc.vector.tensor_add(dst, dst, pst[img][1][:])
                            if last_block:
                                nc.sync.dma_start(out=out[img, :, h0:h0 + R, :],
                                                  in_=dst)
                            elif q == 3:
                                hs = (g // 4) * 16
                                nc.sync.dma_start(out=out[img, :, hs:hs + 16, :],
                                                  in_=st[img][:])
                elif variant == "q32s":
                    # bisect probe: 4 row-quarter positions with SEPARATE
                    # psum banks per c-half, merged by DVE copy + add.
                    for g in range(G):
                        q = g % 4
                        if q == 0:
                            st = [spool.tile([128, 4 * R * Wd], bf16, tag="st",
                                             name=f"st{g}_{i}") for i in range(NIMG)]
                        pst = [[pspool.tile([128, R * Wd], f32, tag="ps",
                                            name=f"ps{g}_{i}_{h}") for h in range(2)]
                               for i in range(NIMG)]
                        h0 = g * R
                        for t in range(KT):
                            dh, dw = TAPS[t]
                            for img in range(NIMG):
                                for hh in range(2):
                                    b = img * 64 + hh * 32
                                    rhs = xv[b:b + 32, h0 + dh:h0 + dh + R,
                                             dw:dw + Wd]
                                    lhsT = wv[b:b + 32, t, :]
                                    nc.tensor.matmul(pst[img][hh][:], lhsT, rhs,
                                                     start=(t == 0),
                                                     stop=(t == KT - 1),
                                                     tile_position=(b, 0))
                        last_block = g >= G - 4
                        for img in range(NIMG):
                            dst = st[img][:, q * R * Wd:(q + 1) * R * Wd]
                            nc.vector.tensor_copy(dst, pst[img][0][:])
                            nc.vector.tensor_add(dst, dst, pst[img][1][:])
                            if last_block:
                                nc.scalar.dma_start(out=out[img, :, h0:h0 + R, :],
                                                    in_=dst)
                            elif q == 3:
                                hs = (g // 4) * 16
                                nc.scalar.dma_start(out=out[img, :, hs:hs + 16, :],
                                                    in_=st[img][:])
                elif variant in ("wr4", "wr4d"):
                    # chain the matmuls with no-sync (ordering-only) edges so
                    # the tile scheduler keeps the weight-reuse emission order
                    prev_mm = [None]

                    def cmm(*args, **kw):
                        bi = nc.tensor.matmul(*args, **kw)
                        if prev_mm[0] is not None:
                            bi.ins.add_dependency(
                                prev_mm[0], mybir.DependencyInfo.NO_SYNC_ONLY)
                        prev_mm[0] = bi.ins.name
                        return bi

                    # supergroups of 4 row-groups; per (img, tap) the weight
                    # tile feeds 4 matmuls (one per group) interleaved across
                    # the two images/positions. With the ldweights dedup pass
                    # (wr4d) each position reloads weights once per tap
                    # instead of 4x: duty 2048/2112 vs 512/576.
                    for sg in range(G // 4):
                        g0 = sg * 4
                        st = [spool.tile([128, 4 * R * Wd], bf16, tag="st",
                                         name=f"st{g0}_{i}") for i in range(NIMG)]
                        pst = [[pspool.tile([128, R * Wd], f32, tag="ps",
                                            name=f"ps{g0}_{i}_{b}") for b in range(4)]
                               for i in range(NIMG)]
                        for t in range(KT):
                            for b in range(4):
                                for img in range(NIMG):
                                    cmm(pst[img][b][:],
                                        lhs_ap(img, t),
                                        rhs_ap(img, g0 + b, t),
                                        start=(t == 0),
                                        stop=(t == KT - 1))
                        last_sg = sg == G // 4 - 1
                        for img in range(NIMG):
                            for b in range(4):
                                evac(g0 + b, b, img, pst[img][b], last_sg)
                elif variant == "dc6":
                    # dual-copy d128 weight-stationary: partitions 0-63 hold
                    # the padded plane (copyA), 64-127 the same plane shifted
                    # down one row (copyB), so a single 128-deep matmul at AP
                    # offset (h0+dh, dw) contracts taps (dh,dw) and (dh+1,dw)
                    # for 64 channels each. Phases p0-2 pair dh=0&1 for the 3
                    # dw; p3-5 read offset h0+1 with zero weights on rows 0-63
                    # to hit taps (2,dw). All matmuls run at tile (0,0) with
                    # one weight phase feeding 8 banks (4 groups x 2 imgs), so
                    # the ldweights dedup pass keeps 6 loads per supergroup.
                    prev_mm = [None]

                    def cmm(*args, **kw):
                        bi = nc.tensor.matmul(*args, **kw)
                        if prev_mm[0] is not None:
                            bi.ins.add_dependency(
                                prev_mm[0], mybir.DependencyInfo.NO_SYNC_ONLY)
                        prev_mm[0] = bi.ins.name
                        return bi

                    for sg in range(G // 4):
                        g0 = sg * 4
                        st = [spool.tile([128, 4 * R * Wd], bf16, tag="st",
                                         name=f"st{g0}_{i}") for i in range(NIMG)]
                        pst = [[pspool.tile([128, R * Wd], f32, tag="ps",
                                            name=f"ps{g0}_{i}_{b}") for b in range(4)]
                               for i in range(NIMG)]
                        for p in range(6):
                            dh0, dw = (0, p) if p < 3 else (1, p - 3)
                            lhsT = wv[:, p, :]
                            for b in range(4):
                                h0 = (g0 + b) * R
                                for img in range(NIMG):
                                    rhs = xv2[img][:, h0 + dh0:h0 + dh0 + R,
                                                   dw:dw + Wd]
                                    cmm(pst[img][b][:], lhsT, rhs,
                                        start=(p == 0), stop=(p == 5),
                                        tile_position=(0, 0))
                        last_sg = sg == G // 4 - 1
                        for img in range(NIMG):
                            for b in range(4):
                                evac(g0 + b, b, img, pst[img][b], last_sg)
                elif variant in ("base", "pemm", "nost", "nold"):
                    # tap-major within a 4-row group; one PSUM bank per
                    # (group, image); 8-bank rotation = 4 groups in flight.
                    for g in range(G):
                        q = g % 4
                        if q == 0:
                            st = [spool.tile([128, 4 * R * Wd], bf16, tag="st",
                                             name=f"st{g}_{i}") for i in range(NIMG)]
                        pst = [pspool.tile([128, R * Wd], f32, tag="ps",
                                           name=f"ps{g}_{i}") for i in range(NIMG)]
                        for t in range(KT):
                            for img in range(NIMG):
                                nc.tensor.matmul(pst[img][:], lhs_ap(img, t),
                                                 rhs_ap(img, g, t),
                                                 start=(t == 0), stop=(t == KT - 1))
                        last_block = g >= G - 4
                        for img in range(NIMG):
                            evac(g, q, img, pst[img], last_block)
                else:
                    # "wr2" / "wr2c": supergroups of 2 row-groups; each
                    # (img, tap) weight tile feeds 2 matmuls back-to-back
                    # (wr2) or alternating with the other image (wr2c).
                    for sg in range(G // 2):
                        g0 = sg * 2
                        q0 = g0 % 4
                        if q0 == 0:
                            st = [spool.tile([128, 4 * R * Wd], bf16, tag="st",
                                             name=f"st{g0}_{i}") for i in range(NIMG)]
                        # pst[img][b] for b in 0..1
                        pst = [[pspool.tile([128, R * Wd], f32, tag="ps",
                                            name=f"ps{g0}_{i}_{b}") for b in range(2)]
                               for i in range(NIMG)]
                        for t in range(KT):
                            if variant == "wr2":
                                order = [(0, 0), (0, 1), (1, 0), (1, 1)]
                            else:  # wr2c: alternate tile positions
                                order = [(0, 0), (1, 0), (0, 1), (1, 1)]
                            for img, b in order:
                                nc.tensor.matmul(pst[img][b][:], lhs_ap(img, t),
                                                 rhs_ap(img, g0 + b, t),
                                                 start=(t == 0), stop=(t == KT - 1))
                        last_block = g0 >= G - 4
                        for img in range(NIMG):
                            for b in range(2):
                                evac(g0 + b, q0 + b, img, pst[img][b], last_block)

            if rep == 1:
                body()
            else:
                with tc.For_i(0, rep, 1, hint_engines=(mybir.EngineType.PE,)):
                    body()
    if variant.endswith("d"):
        n = _dedup_ldweights(nc, mybir)
        assert n > 0, "ldweights dedup removed nothing"
    nc.compile()
    return nc


def _get_nc(rep=1, variant=None):
    key = (rep, variant)
    if key not in _CACHE:
        _CACHE[key] = _build(rep, variant)
    return _CACHE[key]


def _prep_maps(x, weights):
    import ml_dtypes
    bf16 = ml_dtypes.bfloat16
    x = np.ascontiguousarray(x, dtype=np.float32)
    w = np.ascontiguousarray(weights, dtype=np.float32)
    w_t = np.ascontiguousarray(w.transpose(1, 2, 0)).reshape(C, KT * O)
    wp = np.concatenate([w_t, w_t], axis=0).astype(bf16)
    B = x.shape[0]
    xpad = np.zeros((B, C, Wp, Wp), np.float32)
    xpad[:, :, 1:1 + H, 1:1 + Wd] = x
    xpad = xpad.astype(bf16)
    maps = []
    for c in range(NCORES):
        xs = xpad[c * NIMG:(c + 1) * NIMG]
        maps.append({"xp": np.ascontiguousarray(xs).reshape(NIMG * C, Wp * Wp),
                     "wp": wp})
    return maps


def kernel(x, weights):
    from concourse.bass_utils import run_bass_kernel_spmd

    nc = _get_nc()
    maps = _prep_maps(x, weights)
    res = run_bass_kernel_spmd(nc, maps, list(range(NCORES)))
    return np.concatenate([res.results[c]["out"] for c in range(NCORES)],
                          axis=0).astype(np.float32)

